# revision 7
# baseline (speedup 1.0000x reference)
"""Cosine-similarity kernel for trn2: out = l2norm_rows(x) @ l2norm_rows(W).

x: [65536, 512] f32, W: [512, 462] f32 -> out: [65536, 462] f32.

Strategy (data-parallel over 8 cores, batch-sharded x, replicated W):
  Host marshals each core's x shard to x^T fp16 [512, 8192] (layout +
  dtype only; all math on device). fp16 halves both HBM-in and, via an
  fp16 output (host upcasts), HBM-out, and runs the PE at 1 cycle/row
  (fp32r is 2-4x slower). fp16 keeps the result ~3.7e-4 frobenius
  error, far under the 2e-2 gate; all values are O(30) so fp16 range
  is safe.

  Per core (8192 batch rows, 8 groups of 1024 = 8 batch-tiles of 128):
  - GEMM per batch-tile: stationary = x^T chunk [128k, 128b], moving =
    wn chunk [128k, 462] -> PSUM accumulates out in NATURAL layout
    [128b, 462] over 4 k-chunks (one bank; full 128x128 array). The
    LDWEIGHTS pull-ahead hides the per-tile weight swap; matmuls
    stream at N/2.4GHz (~195ns).
  - Row sumsq: GPSIMD (otherwise idle) squares x^T, DVE adds the 4
    k-chunks, then one N=1 matmul per batch-tile with the summed
    square tile as STATIONARY (rhs=ones[128,1]) lands ssq directly on
    per-batch partitions (N=1 matmuls pipeline at ~25ns). ACT Sqrt
    (eps fused via bias) + DVE reciprocal on [128, 8] give the scales;
    no broadcast matmuls, no [128,512] reciprocals, no transposes.
  - Scale+evict fused with PSUM->SBUF copy: 6 tiles on ACT (Copy with
    per-partition scale AP), 2 on DVE, so neither engine blocks PE.
  - A leading dummy Sqrt pins the single ACT table set
    (sqrt_and_others covers Sqrt+Square+Copy) -> one ACT_TABLE_LOAD.
  - Software pipeline: iteration g issues squares(g+1) [GPSIMD] ->
    gemm+scales(g) [PE/ACT/DVE] -> adds+ssqMM+rsqrt(g+1), with x(g+2)
    prefetching; per-engine FIFOs never head-of-line block. Out-DMA
    per half-group shortens the drain tail.
"""

from contextlib import ExitStack

import numpy as np

import concourse.bass as bass
import concourse.mybir as mybir
import concourse.tile as tile
from concourse import bacc, bass_utils
from concourse.bass import ds

N_CORES = 8
B = 65536
B_PER = B // N_CORES          # 8192 batch rows per core
IN_DIM = 512
OUT_DIM = 462
EPS = 1e-12
P = 128
KC = IN_DIM // P              # 4 contraction chunks
GROUP = 1024                  # batch rows per group
TPG = GROUP // P              # 8 batch-tiles per group
HPG = TPG // 2                # half-group tiles (out-DMA granularity)
N_GROUPS = B_PER // GROUP     # 8

F32 = mybir.dt.float32
F16 = mybir.dt.float16
AF = mybir.ActivationFunctionType


def _build_bass():
    nc = bacc.Bacc("TRN2", debug=False, num_devices=N_CORES)
    xt_d = nc.dram_tensor("xt", [IN_DIM, B_PER], F16, kind="ExternalInput").ap()
    w_d = nc.dram_tensor("w", [IN_DIM, OUT_DIM], F32, kind="ExternalInput").ap()
    o_d = nc.dram_tensor("o", [B_PER, OUT_DIM], F16, kind="ExternalOutput").ap()

    with ExitStack() as ctx:
        tc = ctx.enter_context(tile.TileContext(nc))

        # two pools total (every extra pool costs a teardown barrier round);
        # per-tile bufs= overrides give each tile name its own ring depth
        sb = ctx.enter_context(tc.tile_pool(name="sb", bufs=2))
        pp = ctx.enter_context(tc.tile_pool(name="pp", bufs=2, space="PSUM"))

        eps_bias = sb.tile([P, 1], F32, bufs=1)
        nc.vector.memset(eps_bias, EPS)
        ones_mv = sb.tile([P, 1], F16, bufs=1)
        nc.vector.memset(ones_mv, 1.0)
        # table-set pin: first activation is a Sqrt so walrus loads
        # sqrt_and_others (has Sqrt+Square+Copy) once, instead of a
        # square-set first and a 1.3us mid-prologue switch for Sqrt.
        tspin = sb.tile([P, 1], F32, bufs=1)
        nc.scalar.activation(out=tspin, in_=eps_bias, func=AF.Sqrt)

        # ---- W normalization (once): wn = W * rsqrt(rowsumsq + eps) ----
        # per-chunk DMA so Square(c) starts as soon as chunk c lands; this
        # chain gates the first GEMM matmul, so it leads the sync queue.
        w_v = w_d.rearrange("(c p) o -> p c o", p=P)
        w_sb = sb.tile([P, KC, OUT_DIM], F32, bufs=1)
        for c in range(KC):
            nc.sync.dma_start(w_sb[:, c, :], w_v[:, c, :])

        xt_v = xt_d.rearrange("(c p) b -> p c b", p=P)  # [128, KC, B_PER]

        def load_group(g, engine=None):
            x_sb = sb.tile([P, KC, GROUP], F16, bufs=3, name="x_sb")
            (engine or nc.sync).dma_start(x_sb, xt_v[:, :, ds(g * GROUP, GROUP)])
            return x_sb

        # x0 rides the (otherwise idle in the prologue) scalar HWDGE queue
        # so it streams concurrently with W and lands before wn is ready.
        x_tiles = {0: load_group(0, engine=nc.scalar)}

        wsq = sb.tile([P, KC, OUT_DIM], F32, bufs=1)  # scratch squares
        wssq = sb.tile([P, KC], F32, bufs=1)
        for c in range(KC):
            nc.scalar.activation(
                out=wsq[:, c, :],
                in_=w_sb[:, c, :],
                func=AF.Square,
                accum_out=wssq[:, c : c + 1],
            )
        wsd = sb.tile([P, KC], F32, bufs=1)
        nc.scalar.activation(out=wsd, in_=wssq, func=AF.Sqrt, bias=eps_bias)

        x_tiles[1] = load_group(1)

        def ssq_pre(x_sb):
            """squares on GPSIMD -- the idle engine; keeps DVE/ACT clear."""
            xsq = sb.tile([P, KC, GROUP], F16, name="xsq")
            with nc.allow_low_precision(reason="fp16 squares, ssq err ~1e-4"):
                nc.gpsimd.tensor_mul(xsq, x_sb, x_sb)
            return xsq

        def ssq_mid(xsq):
            with nc.allow_low_precision(reason="fp16 adds"):
                t0 = sb.tile([P, GROUP], F16, name="t0")
                t1 = sb.tile([P, GROUP], F16, name="t1")
                xsqs = sb.tile([P, GROUP], F16, name="xsqs")
                nc.vector.tensor_add(t0, xsq[:, 0, :], xsq[:, 1, :])
                nc.vector.tensor_add(t1, xsq[:, 2, :], xsq[:, 3, :])
                nc.vector.tensor_add(xsqs, t0, t1)
            return xsqs

        def ssq_post(xsqs):
            ps_s = pp.tile([P, TPG], F32, name="ps_s")
            for t in range(TPG):
                # stationary = xsqs tile [128k, 128b], moving = ones -> the
                # partition reduce lands ssq on batch partitions directly.
                nc.tensor.matmul(
                    ps_s[:, t : t + 1],
                    lhsT=xsqs[:, ds(t * P, P)],
                    rhs=ones_mv,
                    start=True,
                    stop=True,
                )
            sd = sb.tile([P, TPG], F32, name="sd")
            nc.scalar.activation(out=sd, in_=ps_s, func=AF.Sqrt, bias=eps_bias)
            s_nat = sb.tile([P, TPG], F32, name="s_nat")
            nc.vector.reciprocal(s_nat, sd)
            return s_nat

        xsq0 = ssq_pre(x_tiles[0])

        # wn scales on DVE (tensor_scalar 2x_1P, ~300ns each) right after
        # the reciprocal; ACT stays on the W-squares path.
        wrs = sb.tile([P, KC], F32, bufs=1)
        nc.vector.reciprocal(wrs, wsd)
        wn = sb.tile([P, KC, OUT_DIM], F16, bufs=1)
        with nc.allow_low_precision(reason="fp16 GEMM"):
            for c in range(KC):
                nc.vector.tensor_scalar_mul(wn[:, c, :], w_sb[:, c, :], wrs[:, c : c + 1])

        s_tiles = {0: ssq_post(ssq_mid(xsq0))}

        def gemm_group(g, x_sb, s_nat):
            ot = sb.tile([P, TPG, OUT_DIM], F16, name="ot")
            for t in range(TPG):
                po = pp.tile([P, OUT_DIM], F32, bufs=6, name="po")
                for c in range(KC):
                    nc.tensor.matmul(
                        po,
                        lhsT=x_sb[:, c, ds(t * P, P)],
                        rhs=wn[:, c, :],
                        start=(c == 0),
                        stop=(c == KC - 1),
                    )
                # fused scale-by-rsqrt + PSUM->SBUF evict; mostly on ACT
                # (DVE carries the adds of the next group's ssq chain)
                with nc.allow_low_precision(reason="fp16 out, host upcasts"):
                    if t % 4 == 0:
                        nc.vector.tensor_scalar_mul(ot[:, t, :], po, s_nat[:, t : t + 1])
                    else:
                        nc.scalar.activation(
                            out=ot[:, t, :], in_=po, func=AF.Copy,
                            scale=s_nat[:, t : t + 1],
                        )
                if t % HPG == HPG - 1:
                    # store each half-group as soon as its evicts are issued;
                    # halves the exposed drain on the last group
                    h = t // HPG
                    dst = bass.AP(
                        tensor=o_d.tensor,
                        offset=(g * GROUP + h * HPG * P) * OUT_DIM,
                        ap=[[OUT_DIM, P], [P * OUT_DIM, HPG], [1, OUT_DIM]],
                    )
                    nc.scalar.dma_start(dst, ot[:, h * HPG : (h + 1) * HPG, :])

        # ---- software-pipelined main loop ----
        for g in range(N_GROUPS):
            if g + 2 < N_GROUPS:
                x_tiles[g + 2] = load_group(g + 2)
            xsq_n = ssq_pre(x_tiles[g + 1]) if g + 1 < N_GROUPS else None
            gemm_group(g, x_tiles[g], s_tiles[g])
            del x_tiles[g], s_tiles[g]
            if xsq_n is not None:
                s_tiles[g + 1] = ssq_post(ssq_mid(xsq_n))

    nc.compile()
    return nc


_NC_CACHE = None
LAST_RESULTS = None  # BassKernelResults of the most recent run (for profiling)


def kernel(x: np.ndarray, W: np.ndarray) -> np.ndarray:
    global _NC_CACHE, LAST_RESULTS
    if _NC_CACHE is None:
        _NC_CACHE = _build_bass()
    nc = _NC_CACHE

    x = np.asarray(x, dtype=np.float32)
    W = np.ascontiguousarray(np.asarray(W, dtype=np.float32))
    in_maps = []
    for i in range(N_CORES):
        shard = x[i * B_PER : (i + 1) * B_PER].T.astype(np.float16)  # C-contig
        in_maps.append({"xt": shard, "w": W})
    res = bass_utils.run_bass_kernel_spmd(nc, in_maps, core_ids=list(range(N_CORES)))
    LAST_RESULTS = res
    out = np.concatenate([r["o"] for r in res.results], axis=0).astype(np.float32)
    return out


# revision 10
# speedup vs baseline: 1.4121x; 1.4121x over previous
"""Cosine-similarity kernel for trn2: out = l2norm_rows(x) @ l2norm_rows(W).

x: [65536, 512] f32, W: [512, 462] f32 -> out: [65536, 462] f32.

Strategy (data-parallel over 8 cores, batch-sharded x, replicated W):
  Host marshals each core's x shard to x^T fp16 [512, 8192] (layout +
  dtype only; all math on device). fp16 halves both HBM-in and, via an
  fp16 output (host upcasts), HBM-out, and runs the PE at 1 cycle/row
  (fp32r is 2-4x slower). fp16 keeps the result ~3.7e-4 frobenius
  error, far under the 2e-2 gate; all values are O(30) so fp16 range
  is safe.

  Per core (8192 batch rows, 8 groups of 1024 = 8 batch-tiles of 128):
  - GEMM per batch-tile: stationary = x^T chunk [128k, 128b], moving =
    wn chunk [128k, 462] -> PSUM accumulates out in NATURAL layout
    [128b, 462] over 4 k-chunks (one bank; full 128x128 array). The
    LDWEIGHTS pull-ahead hides the per-tile weight swap; matmuls
    stream at N/2.4GHz (~195ns).
  - Row sumsq: DVE squares x^T (fp16 2x mode), adds the 4 k-chunks,
    then one N=1 matmul per batch-tile with the summed
    square tile as STATIONARY (rhs=ones[128,1]) lands ssq directly on
    per-batch partitions (N=1 matmuls pipeline at ~25ns). ACT Sqrt
    (eps fused via bias) + DVE reciprocal on [128, 8] give the scales;
    no broadcast matmuls, no [128,512] reciprocals, no transposes.
  - Scale+evict fused with PSUM->SBUF copy: 6 tiles on ACT (Copy with
    per-partition scale AP), 2 on DVE, so neither engine blocks PE.
  - A leading dummy Sqrt pins the single ACT table set
    (sqrt_and_others covers Sqrt+Square+Copy) -> one ACT_TABLE_LOAD.
  - Software pipeline: iteration g issues squares+adds(g+1) [DVE] ->
    gemm+scales(g) [PE/ACT/DVE] -> ssqMM+rsqrt(g+1), with x(g+2)
    prefetching; per-engine FIFOs never head-of-line block. Out-DMA
    per half-group shortens the drain tail.
"""

from contextlib import ExitStack

import numpy as np

import concourse.bass as bass
import concourse.mybir as mybir
import concourse.tile as tile
from concourse import bacc, bass_utils
from concourse.bass import ds

N_CORES = 8
B = 65536
B_PER = B // N_CORES          # 8192 batch rows per core
IN_DIM = 512
OUT_DIM = 462
EPS = 1e-12
P = 128
KC = IN_DIM // P              # 4 contraction chunks
GROUP = 1024                  # batch rows per group
TPG = GROUP // P              # 8 batch-tiles per group
HPG = TPG // 2                # half-group tiles (out-DMA granularity)
N_GROUPS = B_PER // GROUP     # 8

F32 = mybir.dt.float32
F16 = mybir.dt.float16
AF = mybir.ActivationFunctionType


def _build_bass():
    nc = bacc.Bacc("TRN2", debug=False, num_devices=N_CORES)
    xt_d = nc.dram_tensor("xt", [IN_DIM, B_PER], F16, kind="ExternalInput").ap()
    w_d = nc.dram_tensor("w", [IN_DIM, OUT_DIM], F32, kind="ExternalInput").ap()
    o_d = nc.dram_tensor("o", [B_PER, OUT_DIM], F16, kind="ExternalOutput").ap()

    with ExitStack() as ctx:
        tc = ctx.enter_context(tile.TileContext(nc))

        # two pools total (every extra pool costs a teardown barrier round);
        # per-tile bufs= overrides give each tile name its own ring depth
        sb = ctx.enter_context(tc.tile_pool(name="sb", bufs=2))
        pp = ctx.enter_context(tc.tile_pool(name="pp", bufs=2, space="PSUM"))

        eps_bias = sb.tile([P, 1], F32, bufs=1)
        nc.vector.memset(eps_bias, EPS)
        ones_mv = sb.tile([P, 1], F16, bufs=1)
        nc.vector.memset(ones_mv, 1.0)
        # table-set pin: first activation is a Sqrt so walrus loads
        # sqrt_and_others (has Sqrt+Square+Copy) once, instead of a
        # square-set first and a 1.3us mid-prologue switch for Sqrt.
        tspin = sb.tile([P, 1], F32, bufs=1)
        nc.scalar.activation(out=tspin, in_=eps_bias, func=AF.Sqrt)

        # ---- W normalization (once): wn = W * rsqrt(rowsumsq + eps) ----
        # per-chunk DMA so Square(c) starts as soon as chunk c lands; this
        # chain gates the first GEMM matmul, so it leads the sync queue.
        w_v = w_d.rearrange("(c p) o -> p c o", p=P)
        w_sb = sb.tile([P, KC, OUT_DIM], F32, bufs=1)
        for c in range(KC):
            nc.sync.dma_start(w_sb[:, c, :], w_v[:, c, :])

        xt_v = xt_d.rearrange("(c p) b -> p c b", p=P)  # [128, KC, B_PER]

        def load_group(g, engine=None):
            x_sb = sb.tile([P, KC, GROUP], F16, bufs=3, name="x_sb")
            (engine or nc.sync).dma_start(x_sb, xt_v[:, :, ds(g * GROUP, GROUP)])
            return x_sb

        # x0 rides the (otherwise idle in the prologue) scalar HWDGE queue
        # so it streams concurrently with W and lands before wn is ready.
        x_tiles = {0: load_group(0, engine=nc.scalar)}

        wsq = sb.tile([P, KC, OUT_DIM], F32, bufs=1)  # scratch squares
        wssq = sb.tile([P, KC], F32, bufs=1)
        for c in range(KC):
            nc.scalar.activation(
                out=wsq[:, c, :],
                in_=w_sb[:, c, :],
                func=AF.Square,
                accum_out=wssq[:, c : c + 1],
            )
        wsd = sb.tile([P, KC], F32, bufs=1)
        nc.scalar.activation(out=wsd, in_=wssq, func=AF.Sqrt, bias=eps_bias)

        x_tiles[1] = load_group(1)

        def ssq_pre(x_sb):
            """DVE fp16 2x-mode squares (~2.2us/group). GPSIMD tensor_mul was
            tried and is 3x slower AND wrecks DVE via SBUF-port contention."""
            xsq = sb.tile([P, KC, GROUP], F16, name="xsq")
            with nc.allow_low_precision(reason="fp16 squares, ssq err ~1e-4"):
                nc.vector.tensor_mul(xsq, x_sb, x_sb)
            return xsq

        def ssq_mid(xsq):
            with nc.allow_low_precision(reason="fp16 adds"):
                t0 = sb.tile([P, GROUP], F16, name="t0")
                t1 = sb.tile([P, GROUP], F16, name="t1")
                xsqs = sb.tile([P, GROUP], F16, name="xsqs")
                nc.vector.tensor_add(t0, xsq[:, 0, :], xsq[:, 1, :])
                nc.vector.tensor_add(t1, xsq[:, 2, :], xsq[:, 3, :])
                nc.vector.tensor_add(xsqs, t0, t1)
            return xsqs

        def ssq_post(xsqs):
            ps_s = pp.tile([P, TPG], F32, bufs=1, name="ps_s")
            for t in range(TPG):
                # stationary = xsqs tile [128k, 128b], moving = ones -> the
                # partition reduce lands ssq on batch partitions directly.
                nc.tensor.matmul(
                    ps_s[:, t : t + 1],
                    lhsT=xsqs[:, ds(t * P, P)],
                    rhs=ones_mv,
                    start=True,
                    stop=True,
                )
            sd = sb.tile([P, TPG], F32, name="sd")
            nc.scalar.activation(out=sd, in_=ps_s, func=AF.Sqrt, bias=eps_bias)
            s_nat = sb.tile([P, TPG], F32, name="s_nat")
            nc.vector.reciprocal(s_nat, sd)
            return s_nat

        # wn scales on DVE (tensor_scalar 2x_1P, ~300ns each) right after
        # the reciprocal, AHEAD of group-0 squares in the DVE FIFO, so the
        # first GEMM matmul is gated only by the short W chain.
        wrs = sb.tile([P, KC], F32, bufs=1)
        nc.vector.reciprocal(wrs, wsd)
        wn = sb.tile([P, KC, OUT_DIM], F16, bufs=1)
        with nc.allow_low_precision(reason="fp16 GEMM"):
            for c in range(KC):
                nc.vector.tensor_scalar_mul(wn[:, c, :], w_sb[:, c, :], wrs[:, c : c + 1])

        s_tiles = {0: ssq_post(ssq_mid(ssq_pre(x_tiles[0])))}

        def gemm_group(g, x_sb, s_nat):
            ot = sb.tile([P, TPG, OUT_DIM], F16, name="ot")
            for t in range(TPG):
                po = pp.tile([P, OUT_DIM], F32, bufs=7, name="po")
                for c in range(KC):
                    nc.tensor.matmul(
                        po,
                        lhsT=x_sb[:, c, ds(t * P, P)],
                        rhs=wn[:, c, :],
                        start=(c == 0),
                        stop=(c == KC - 1),
                    )
                # fused scale-by-rsqrt + PSUM->SBUF evict; mostly on ACT
                # (DVE carries the adds of the next group's ssq chain)
                with nc.allow_low_precision(reason="fp16 out, host upcasts"):
                    if t % 4 == 3:
                        nc.vector.tensor_scalar_mul(ot[:, t, :], po, s_nat[:, t : t + 1])
                    else:
                        nc.scalar.activation(
                            out=ot[:, t, :], in_=po, func=AF.Copy,
                            scale=s_nat[:, t : t + 1],
                        )
                # store each half-group as soon as its evicts are issued;
                # on the kernel's final tiles use smaller stores (the last
                # DMA's completion receipt is exposed, so keep it small):
                # last group stores as [0-3], [4-5], [6], [7].
                if g == N_GROUPS - 1 and t >= HPG:
                    spans = {5: (4, 2), 6: (6, 1), 7: (7, 1)}
                    t0n = spans.get(t)
                else:
                    t0n = (t - HPG + 1, HPG) if t % HPG == HPG - 1 else None
                if t0n is not None:
                    t0_, n_t = t0n
                    dst = bass.AP(
                        tensor=o_d.tensor,
                        offset=(g * GROUP + t0_ * P) * OUT_DIM,
                        ap=[[OUT_DIM, P], [P * OUT_DIM, n_t], [1, OUT_DIM]],
                    )
                    nc.scalar.dma_start(dst, ot[:, t0_ : t0_ + n_t, :])

        # ---- software-pipelined main loop ----
        for g in range(N_GROUPS):
            if g + 2 < N_GROUPS:
                x_tiles[g + 2] = load_group(g + 2)
            # squares+adds(g+1) lead the DVE FIFO (their x landed last
            # iteration), so the g+1 ssq matmuls are ready the moment the
            # PE drains gemm(g) -- no group-boundary stall; the two DVE
            # scale-evicts of group g queue behind them, which is fine
            # since ACT covers the early tiles.
            xsqs_n = ssq_mid(ssq_pre(x_tiles[g + 1])) if g + 1 < N_GROUPS else None
            gemm_group(g, x_tiles[g], s_tiles[g])
            del x_tiles[g], s_tiles[g]
            if xsqs_n is not None:
                s_tiles[g + 1] = ssq_post(xsqs_n)

    nc.compile()
    return nc


_NC_CACHE = None
LAST_RESULTS = None  # BassKernelResults of the most recent run (for profiling)


def kernel(x: np.ndarray, W: np.ndarray) -> np.ndarray:
    global _NC_CACHE, LAST_RESULTS
    if _NC_CACHE is None:
        _NC_CACHE = _build_bass()
    nc = _NC_CACHE

    x = np.asarray(x, dtype=np.float32)
    W = np.ascontiguousarray(np.asarray(W, dtype=np.float32))
    in_maps = []
    for i in range(N_CORES):
        shard = x[i * B_PER : (i + 1) * B_PER].T.astype(np.float16)  # C-contig
        in_maps.append({"xt": shard, "w": W})
    res = bass_utils.run_bass_kernel_spmd(nc, in_maps, core_ids=list(range(N_CORES)))
    LAST_RESULTS = res
    out = np.concatenate([r["o"] for r in res.results], axis=0).astype(np.float32)
    return out
